# revision 1
# baseline (speedup 1.0000x reference)
"""Trainium2 Bass kernel for nn_GaussianBasis (2D gaussian-splat sum rasterizer).

Math: out[c,d,h,w] = sum_n opacity_n * exp(-sigma_n(h,w)) * features[c,n,d]
where sigma is a per-gaussian quadratic form in pixel coords.

Strategy:
  - Gaussians have tiny support (std <= ~1.8px, 6-sigma radius <= ~11px), so
    bin them host-side into 32x32-pixel buckets (8 h-bands x 8 w-cols) with a
    sigma <= SIG_CUT cutoff ellipse; contributions outside vanish in fp32.
  - sigma over a bucket is a K=6 matmul: sigma[k,px] = W6[:,k]^T @ phi[:,px],
    phi = [x^2, y^2, x*y, x, y, 1] in bucket-CENTERED coords. With |x|,|y| <=
    15.5 every phi entry is a quarter-integer <= 240.25 — exactly
    representable in fp16. W6 is split hi/lo into two fp16 halves and both
    matmuls fold into ONE K=12 fp16 matmul (1 cycle/row on PE vs 4 for fp32).
  - Each of the 8 cores owns one 32-row h-band: per col-bucket, PE computes
    sigma (K=12 fp16 matmul, fp32 PSUM) -> ACT computes g=exp(-sigma)
    PSUM->SBUF (fp16 out, 1024px per instr) -> PE computes the feature einsum
    (fp16 matmul, fp32 PSUM accumulate) -> DMA the PSUM accumulator straight
    to the output band. No collectives: pixel-sharding keeps outputs disjoint.
"""

import sys
import os

sys.path.insert(0, "/opt/trn_rl_repo")

import numpy as np
from contextlib import ExitStack

N, C, H, W = 2048, 16, 256, 256
NCORES = 8
BH, BW = 32, 32               # bucket (tile) size in pixels
NBH, NBW = H // BH, W // BW   # 8 h-bands (one per core), 8 w-cols
PX = BH * BW                  # 1024 pixels per bucket
CHUNK = 512                   # pixels per matmul (one PSUM bank of fp32 out)
NCH = PX // CHUNK             # 2 chunks per bucket
SIG_CUT = 18.0                # exp(-18) ~ 1.5e-8: negligible vs output scale

_cached = {}


def _host_prep(xyz_raw, cholesky_raw, features, opacity):
    """Bin gaussians into (band, col) buckets; emit per-bucket quadratic
    coefficients (bucket-centered coords, fp16 hi/lo split) and
    opacity-folded feature matrices."""
    xy = np.tanh(xyz_raw.astype(np.float64))
    cx = 0.5 * (xy[:, 0] + 1.0) * W
    cy = 0.5 * (xy[:, 1] + 1.0) * H
    chol = cholesky_raw.astype(np.float64) + np.array([0.5, 0.0, 0.5])
    l1, l2, l3 = chol[:, 0], chol[:, 1], chol[:, 2]
    a = l1 * l1
    b = l1 * l2
    c = l2 * l2 + l3 * l3
    det = a * c - b * b
    Aq = 0.5 * (c / det)      # coeff of dx^2
    Bq = -b / det             # coeff of dx*dy
    Cq = 0.5 * (a / det)      # coeff of dy^2
    # ellipse {sigma <= SIG_CUT} axis-aligned bounding half-widths
    rx = np.sqrt(2.0 * SIG_CUT * a) + 1.0
    ry = np.sqrt(2.0 * SIG_CUT * c) + 1.0

    featw = features.astype(np.float64) * opacity[:, 0][None, :, None]  # [C,N,3]
    featw = np.transpose(featw, (1, 0, 2)).reshape(N, C * 3)            # [N,48]

    buckets = [[[] for _ in range(NBW)] for _ in range(NBH)]
    h_lo = np.floor(cy - ry).astype(int)
    h_hi = np.ceil(cy + ry).astype(int)
    w_lo = np.floor(cx - rx).astype(int)
    w_hi = np.ceil(cx + rx).astype(int)
    for n in range(N):
        for bh in range(max(0, h_lo[n] // BH), min(NBH, h_hi[n] // BH + 1)):
            for bw in range(max(0, w_lo[n] // BW), min(NBW, w_hi[n] // BW + 1)):
                buckets[bh][bw].append(n)

    kmax = max(len(buckets[i][j]) for i in range(NBH) for j in range(NBW))
    NT = max(1, (kmax + 127) // 128)
    K_pad = NT * 128

    # Arrays laid out exactly as the SBUF tiles expect, so each input is ONE
    # contiguous DMA: w12 [12, NBW*K_pad], feat [128, NBW*NT*48].
    w12 = np.zeros((NBH, 12, NBW * K_pad), dtype=np.float16)
    feat = np.zeros((NBH, 128, NBW * NT * 48), dtype=np.float16)
    for bh in range(NBH):
        for bw in range(NBW):
            ns = np.array(buckets[bh][bw], dtype=int)
            k = len(ns)
            if k == 0:
                continue
            cxl = cx[ns] - bw * BW - BW / 2
            cyl = cy[ns] - bh * BH - BH / 2
            An, Bn, Cn = Aq[ns], Bq[ns], Cq[ns]
            W6 = np.stack(
                [
                    An,
                    Cn,
                    Bn,
                    -(2.0 * An * cxl + Bn * cyl),
                    -(2.0 * Cn * cyl + Bn * cxl),
                    An * cxl * cxl + Cn * cyl * cyl + Bn * cxl * cyl,
                ],
                0,
            )
            W_hi = W6.astype(np.float16)
            W_lo = (W6 - W_hi.astype(np.float64)).astype(np.float16)
            w12[bh, :6, bw * K_pad:bw * K_pad + k] = W_hi
            w12[bh, 6:, bw * K_pad:bw * K_pad + k] = W_lo
            fk = featw[ns].astype(np.float16)            # [k, 48]
            for nt in range((k + 127) // 128):
                p = min(128, k - nt * 128)
                feat[bh, :p, (bw * NT + nt) * 48:(bw * NT + nt + 1) * 48] = \
                    fk[nt * 128:nt * 128 + p]

    # bucket-centered pixel coords: every entry a quarter-integer <= 240.25,
    # exact in fp16
    xs = (np.arange(BW) + 0.5 - BW / 2).astype(np.float32)
    ys = (np.arange(BH) + 0.5 - BH / 2).astype(np.float32)
    Yg, Xg = np.meshgrid(ys, xs, indexing="ij")
    phi6 = np.stack(
        [Xg * Xg, Yg * Yg, Xg * Yg, Xg, Yg, np.ones_like(Xg)], 0
    ).reshape(6, PX)
    phi12 = np.concatenate([phi6, phi6], 0).astype(np.float16)  # [12, PX]
    return w12, feat, phi12, NT


def _build_program(NT):
    import concourse.bacc as bacc
    import concourse.tile as tile
    import concourse.mybir as mybir

    nc = bacc.Bacc("TRN2", target_bir_lowering=False, debug=False,
                   num_devices=NCORES)
    KP = NT * 128
    w12_ap = nc.dram_tensor("w12", [12, NBW * KP], mybir.dt.float16,
                            kind="ExternalInput").ap()
    feat_ap = nc.dram_tensor("feat", [128, NBW * NT * 48], mybir.dt.float16,
                             kind="ExternalInput").ap()
    phi_ap = nc.dram_tensor("phi", [12, PX], mybir.dt.float16,
                            kind="ExternalInput").ap()
    out_ap = nc.dram_tensor("out", [C * 3, BH, W], mybir.dt.float32,
                            kind="ExternalOutput").ap()

    HB = BH // NCH  # h-rows per chunk (16)
    with tile.TileContext(nc) as tc:
        with ExitStack() as ctx:
            consts = ctx.enter_context(tc.tile_pool(name="consts", bufs=1))
            spool = ctx.enter_context(
                tc.tile_pool(name="sig", bufs=3, space="PSUM"))
            opool = ctx.enter_context(
                tc.tile_pool(name="acc", bufs=2, space="PSUM"))
            gpool = ctx.enter_context(tc.tile_pool(name="g", bufs=3))

            # PE HAM warmup: dummy matmuls on a zeroed SBUF tile while the
            # input DMAs are in flight, so real matmuls start at 2.4 GHz.
            # They rotate through the same psum_s slots as the real sigma
            # matmuls (same tag), serializing only on PE, which is idle.
            dummy = consts.tile([12, 640], mybir.dt.float16)
            nc.vector.memset(dummy, 0)
            for _ in range(2):
                psum_s = spool.tile([128, PX], mybir.dt.float32)
                nc.tensor.matmul(psum_s[:, 0:CHUNK], dummy[:, 0:128],
                                 dummy[:, 128:640], start=True, stop=True)

            # inputs: one contiguous DMA each; phi+w12 on the SP HWDGE queue
            # (ACT's queue is busy with the exp table load), feat on SWDGE
            phi_sb = consts.tile([12, PX], mybir.dt.float16)
            nc.sync.dma_start(out=phi_sb, in_=phi_ap)
            w12_sb = consts.tile([12, NBW * KP], mybir.dt.float16)
            nc.sync.dma_start(out=w12_sb, in_=w12_ap)
            feat_sb = consts.tile([128, NBW * NT * 48], mybir.dt.float16)
            nc.gpsimd.dma_start(out=feat_sb, in_=feat_ap)

            # final band accumulator in SBUF: partitions [0:48] hold chunk 0
            # (h 0..15), [64:112] chunk 1 (h 16..31); free dim is the DRAM
            # band layout (h-major, w global) so the output DMA is contiguous
            out_sb = consts.tile([112, HB * W], mybir.dt.float32)

            for col in range(NBW):
                psum_o = opool.tile([112, CHUNK], mybir.dt.float32)
                for nt in range(NT):
                    psum_s = spool.tile([128, PX], mybir.dt.float32)
                    for ch in range(NCH):
                        nc.tensor.matmul(
                            psum_s[:, ch * CHUNK:(ch + 1) * CHUNK],
                            w12_sb[:, (col * NT + nt) * 128:(col * NT + nt + 1) * 128],
                            phi_sb[:, ch * CHUNK:(ch + 1) * CHUNK],
                            start=True, stop=True)
                    g = gpool.tile([128, PX], mybir.dt.float16)
                    nc.scalar.activation(
                        g, psum_s, mybir.ActivationFunctionType.Exp,
                        bias=0.0, scale=-1.0)
                    for ch in range(NCH):
                        nc.tensor.matmul(
                            psum_o[64 * ch:64 * ch + 48, :],
                            feat_sb[:, (col * NT + nt) * 48:(col * NT + nt + 1) * 48],
                            g[:, ch * CHUNK:(ch + 1) * CHUNK],
                            start=(nt == 0), stop=(nt == NT - 1),
                            tile_position=(0, 64 * ch))
                nc.vector.tensor_copy(
                    out_sb.rearrange("p (h cw) -> p h cw", cw=W)[
                        :, :, col * BW:(col + 1) * BW],
                    psum_o.rearrange("p (h w) -> p h w", w=BW))

            # two contiguous output DMAs: partitions [0:48] -> h rows 0..15,
            # [64:112] -> h rows 16..31
            for ch in range(NCH):
                nc.sync.dma_start(
                    out=out_ap[:, ch * HB:(ch + 1) * HB, :],
                    in_=out_sb[64 * ch:64 * ch + 48, :].rearrange(
                        "p (h cw) -> p h cw", cw=W))
    nc.compile()
    return nc


def _host_prep_packed(cx, cy, Aq, Bq, Cq, rx, ry, featw):
    """16x16-px buckets, two vertical halves packed per 128-partition tile
    (top half-band -> partitions 0:64, bottom -> 64:128). Requires every
    bucket to hold <= 64 gaussians; returns None if not."""
    BH2 = BW2 = 16
    ncol = W // BW2                       # 16 cols per band
    nrow = H // BH2                       # 16 half-band rows
    buckets = [[[] for _ in range(ncol)] for _ in range(nrow)]
    h_lo = np.floor(cy - ry).astype(int)
    h_hi = np.ceil(cy + ry).astype(int)
    w_lo = np.floor(cx - rx).astype(int)
    w_hi = np.ceil(cx + rx).astype(int)
    for n in range(N):
        for bh in range(max(0, h_lo[n] // BH2), min(nrow, h_hi[n] // BH2 + 1)):
            for bw in range(max(0, w_lo[n] // BW2), min(ncol, w_hi[n] // BW2 + 1)):
                buckets[bh][bw].append(n)
    if max(len(buckets[i][j]) for i in range(nrow) for j in range(ncol)) > 64:
        return None

    PX2 = BH2 * BW2
    w12 = np.zeros((NCORES, 12, PX2 + ncol * 128), dtype=np.float16)
    feat = np.zeros((NCORES, 128, ncol * 48), dtype=np.float16)
    for core in range(NCORES):
        for col in range(ncol):
            for half in range(2):
                ns = np.array(buckets[2 * core + half][col], dtype=int)
                k = len(ns)
                if k == 0:
                    continue
                cxl = cx[ns] - col * BW2 - BW2 / 2
                cyl = cy[ns] - (2 * core + half) * BH2 - BH2 / 2
                An, Bn, Cn = Aq[ns], Bq[ns], Cq[ns]
                W6 = np.stack(
                    [
                        An,
                        Cn,
                        Bn,
                        -(2.0 * An * cxl + Bn * cyl),
                        -(2.0 * Cn * cyl + Bn * cxl),
                        An * cxl * cxl + Cn * cyl * cyl + Bn * cxl * cyl,
                    ],
                    0,
                )
                W_hi = W6.astype(np.float16)
                W_lo = (W6 - W_hi.astype(np.float64)).astype(np.float16)
                base = PX2 + col * 128 + 64 * half
                w12[core, :6, base:base + k] = W_hi
                w12[core, 6:, base:base + k] = W_lo
                feat[core, 64 * half:64 * half + k, col * 48:col * 48 + 48] = \
                    featw[ns].astype(np.float16)

    xs = (np.arange(BW2) + 0.5 - BW2 / 2).astype(np.float32)
    ys = (np.arange(BH2) + 0.5 - BH2 / 2).astype(np.float32)
    Yg, Xg = np.meshgrid(ys, xs, indexing="ij")
    phi6 = np.stack(
        [Xg * Xg, Yg * Yg, Xg * Yg, Xg, Yg, np.ones_like(Xg)], 0
    ).reshape(6, BH2 * BW2)
    phi12 = np.concatenate([phi6, phi6], 0).astype(np.float16)  # [12, 256]
    w12[:, :, 0:PX2] = phi12[None]
    return w12, feat, phi12


def _build_program_packed():
    import concourse.bacc as bacc
    import concourse.tile as tile
    import concourse.mybir as mybir

    BH2 = BW2 = 16
    ncol = W // BW2                 # 16 packed tiles per core
    PX2 = BH2 * BW2                 # 256 px per bucket
    npair = ncol // 2               # col pairs sharing one PSUM/ACT group

    nc = bacc.Bacc("TRN2", target_bir_lowering=False, debug=False,
                   num_devices=NCORES)
    # phi rides in the same tensor as w12 (FIRST PX2 columns), so the first
    # DMA chunk (phi + first 4 col tiles) lands before the rest
    w12_ap = nc.dram_tensor("w12", [12, PX2 + ncol * 128], mybir.dt.float16,
                            kind="ExternalInput").ap()
    feat_ap = nc.dram_tensor("feat", [128, ncol * 48], mybir.dt.float16,
                             kind="ExternalInput").ap()
    out_ap = nc.dram_tensor("out", [C * 3, BH, W], mybir.dt.float32,
                            kind="ExternalOutput").ap()

    with tile.TileContext(nc) as tc:
        with ExitStack() as ctx:
            consts = ctx.enter_context(tc.tile_pool(name="consts", bufs=1))
            spool = ctx.enter_context(
                tc.tile_pool(name="sig", bufs=2, space="PSUM"))
            opool = ctx.enter_context(
                tc.tile_pool(name="acc", bufs=3, space="PSUM"))
            gpool = ctx.enter_context(tc.tile_pool(name="g", bufs=3))

            dummy = consts.tile([12, 640], mybir.dt.float16)
            nc.vector.memset(dummy, 0)
            for _ in range(2):
                psum_s = spool.tile([128, 4 * PX2], mybir.dt.float32)
                nc.tensor.matmul(psum_s[:, 0:512], dummy[:, 0:128],
                                 dummy[:, 128:640], start=True, stop=True)

            w12_sb = consts.tile([12, PX2 + ncol * 128], mybir.dt.float16)
            CUT = PX2 + 4 * 128
            nc.sync.dma_start(out=w12_sb[:, :CUT], in_=w12_ap[:, :CUT])
            nc.sync.dma_start(out=w12_sb[:, CUT:], in_=w12_ap[:, CUT:])
            phi_sb = w12_sb[:, 0:PX2]
            feat_sb = consts.tile([128, ncol * 48], mybir.dt.float16)
            nc.gpsimd.dma_start(out=feat_sb, in_=feat_ap)

            # band accumulator, h-major DRAM layout; partitions [0:48] hold
            # h 0..15, [64:112] h 16..31
            out_sb = consts.tile([112, (BH // 2) * W], mybir.dt.float32)
            out_v = out_sb.rearrange("p (h cw) -> p h cw", cw=W)

            for qr in range(npair // 2):
                # one 4-col sigma/exp group (fewer ACT instruction overheads)
                psum_s = spool.tile([128, 4 * PX2], mybir.dt.float32)
                for j in range(4):
                    t = 4 * qr + j
                    nc.tensor.matmul(
                        psum_s[:, j * PX2:(j + 1) * PX2],
                        w12_sb[:, PX2 + t * 128:PX2 + (t + 1) * 128],
                        phi_sb,
                        start=True, stop=True)
                g = gpool.tile([128, 4 * PX2], mybir.dt.float16)
                nc.scalar.activation(
                    g, psum_s, mybir.ActivationFunctionType.Exp,
                    bias=0.0, scale=-1.0)
                for pq in range(2):
                    pr = 2 * qr + pq
                    psum_o = opool.tile([112, 512], mybir.dt.float32)
                    for j in range(2):
                        t = 2 * pr + j
                        gj = 2 * pq + j
                        for half in range(2):
                            nc.tensor.matmul(
                                psum_o[64 * half:64 * half + 48,
                                       j * PX2:(j + 1) * PX2],
                                feat_sb[64 * half:64 * half + 64,
                                        t * 48:(t + 1) * 48],
                                g[64 * half:64 * half + 64,
                                  gj * PX2:(gj + 1) * PX2],
                                start=True, stop=True,
                                tile_position=(64 * half, 64 * half))
                    # psum free order (c2, h16, w16) -> out (h-major, global w)
                    nc.vector.tensor_copy(
                        out_v[:, :, pr * 2 * BW2:(pr + 1) * 2 * BW2].rearrange(
                            "p h (c w) -> p c h w", w=BW2),
                        psum_o.rearrange("p (c h w) -> p c h w",
                                         h=BH2, w=BW2))

            for ch in range(2):
                nc.sync.dma_start(
                    out=out_ap[:, ch * (BH // 2):(ch + 1) * (BH // 2), :],
                    in_=out_sb[64 * ch:64 * ch + 48, :].rearrange(
                        "p (h cw) -> p h cw", cw=W))
    nc.compile()
    return nc


def _params(np_inputs):
    """Per-gaussian params (fp64 host): centers, quadratic coeffs, cutoff
    radii, opacity-folded features."""
    xyz_raw = np.asarray(np_inputs["xyz_raw"], dtype=np.float32)
    cholesky_raw = np.asarray(np_inputs["cholesky_raw"], dtype=np.float32)
    features = np.asarray(np_inputs["features"], dtype=np.float32)
    opacity = np.asarray(np_inputs["opacity"], dtype=np.float32)
    xy = np.tanh(xyz_raw.astype(np.float64))
    cx = 0.5 * (xy[:, 0] + 1.0) * W
    cy = 0.5 * (xy[:, 1] + 1.0) * H
    chol = cholesky_raw.astype(np.float64) + np.array([0.5, 0.0, 0.5])
    l1, l2, l3 = chol[:, 0], chol[:, 1], chol[:, 2]
    a = l1 * l1
    b = l1 * l2
    c = l2 * l2 + l3 * l3
    det = a * c - b * b
    Aq, Bq, Cq = 0.5 * (c / det), -b / det, 0.5 * (a / det)
    rx = np.sqrt(2.0 * SIG_CUT * a) + 1.0
    ry = np.sqrt(2.0 * SIG_CUT * c) + 1.0
    featw = features.astype(np.float64) * opacity[:, 0][None, :, None]
    featw = np.transpose(featw, (1, 0, 2)).reshape(N, C * 3)
    return cx, cy, Aq, Bq, Cq, rx, ry, featw


def kernel(xyz_raw, cholesky_raw, features, opacity):
    from concourse.bass_utils import run_bass_kernel_spmd

    xyz_raw = np.asarray(xyz_raw, dtype=np.float32)
    cholesky_raw = np.asarray(cholesky_raw, dtype=np.float32)
    features = np.asarray(features, dtype=np.float32)
    opacity = np.asarray(opacity, dtype=np.float32)

    cx, cy, Aq, Bq, Cq, rx, ry, featw = _params({
        "xyz_raw": xyz_raw, "cholesky_raw": cholesky_raw,
        "features": features, "opacity": opacity})

    packed = _host_prep_packed(cx, cy, Aq, Bq, Cq, rx, ry, featw)
    if packed is not None:
        w12, feat, _ = packed
        if "packed" not in _cached:
            _cached["packed"] = _build_program_packed()
        nc = _cached["packed"]
        in_maps = [
            {"w12": w12[band], "feat": feat[band]} for band in range(NCORES)
        ]
    else:
        w12, feat, phi12, NT = _host_prep(
            xyz_raw, cholesky_raw, features, opacity)
        if NT not in _cached:
            _cached[NT] = _build_program(NT)
        nc = _cached[NT]
        in_maps = [
            {"w12": w12[band], "feat": feat[band], "phi": phi12}
            for band in range(NCORES)
        ]
    res = run_bass_kernel_spmd(nc, in_maps, core_ids=list(range(NCORES)))

    out = np.empty((C * 3, H, W), dtype=np.float32)
    for band in range(NCORES):
        out[:, band * BH:(band + 1) * BH, :] = res.results[band]["out"]
    return out.reshape(C, 3, H, W)



# revision 32
# speedup vs baseline: 1.5830x; 1.5830x over previous
"""Trainium2 Bass kernel for nn_GaussianBasis (2D gaussian-splat sum rasterizer).

Math: out[c,d,h,w] = sum_n opacity_n * exp(-sigma_n(h,w)) * features[c,n,d]
where sigma is a per-gaussian quadratic form in pixel coords.

Strategy (v3):
  - Bin gaussians host-side into 8x16-px buckets with a sigma <= SIG_CUT
    cutoff ellipse (SIG_CUT=12 keeps every bucket <= 32 gaussians here;
    bigger buckets are split into two slots over the same pixels — exact,
    since the rasterizer is linear and the host adds partials). Each core
    owns one 32-row band = 64 buckets; four 32-gaussian slots pack one
    128-partition tile -> T = 16 tiles/core, sigma/exp columns = pixels/4.
  - sigma over a bucket is a K=6 matmul vs phi = [x^2,y^2,xy,x,y,1] in
    bucket-centered quarter-integer coords (exact in fp16); W6 split hi/lo
    doubles K to 12 for fp32-grade accuracy at fp16 matmul speed. One
    matmul per tile: psum [128, 128px].
  - Feature einsum: per tile, two stacked matmuls (slot pairs (0,1) rows
    0:64 and (2,3) rows 64:128 via tile_position): lhsT [64, 96] holds slot
    2p's features in rows 0:32 -> cols 0:48 and slot 2p+1's in rows 32:64
    -> cols 48:96, yielding psum [96, 128] per pair.
  - exp on ACT per exp-group (PSUM->SBUF fp16); per-tile PSUM->SBUF fp16
    convert-copies spread across DVE/Pool/ACT; outputs leave via PREPARED
    SWDGE scatter-DMAs (dma_scatter_add with identity indices onto the
    zero-initialized output buffer): descriptors are generated on Pool long
    before the data exists, and a ~40ns trigger fires each transfer the
    moment its copies land — no HWDGE/DGE-delay on the critical tail.
    Output rides as fp16; the host scatters slot blocks and upcasts
    (tol 2e-2 >> fp16 error).
  - PE p-state warmup: dummy matmuls bridge the input-DMA latency so the
    ramp clock starts early.
"""

import sys

sys.path.insert(0, "/opt/trn_rl_repo")

import numpy as np
from contextlib import ExitStack

N, C, H, W = 2048, 16, 256, 256
NCORES = 8
BH = 32                        # band height per core
BKH, BKW = 8, 16               # bucket shape
BPX = BKH * BKW                # 128 px per bucket
NBR = BH // BKH                # 4 bucket rows per core
NBC = W // BKW                 # 16 bucket cols
SLOT = 32                      # gaussians per slot (quarter tile)
SIG_CUT = 12.0                 # exp(-12) ~ 6e-6: negligible vs output scale

# exp-group tile spans (ACT pipeline) and flush-group spans (scatter DMAs);
# built for T=16, recomputed in _build_program for other T
EXP_GROUPS = [2, 4, 4, 4, 2]
FLUSH_GROUPS = [4, 4, 6, 2]

_cached = {}


def _params(np_inputs):
    """Per-gaussian params (fp64 host): centers, quadratic coeffs, cutoff
    radii, opacity-folded features."""
    xyz_raw = np.asarray(np_inputs["xyz_raw"], dtype=np.float32)
    cholesky_raw = np.asarray(np_inputs["cholesky_raw"], dtype=np.float32)
    features = np.asarray(np_inputs["features"], dtype=np.float32)
    opacity = np.asarray(np_inputs["opacity"], dtype=np.float32)
    xy = np.tanh(xyz_raw.astype(np.float64))
    cx = 0.5 * (xy[:, 0] + 1.0) * W
    cy = 0.5 * (xy[:, 1] + 1.0) * H
    chol = cholesky_raw.astype(np.float64) + np.array([0.5, 0.0, 0.5])
    l1, l2, l3 = chol[:, 0], chol[:, 1], chol[:, 2]
    a = l1 * l1
    b = l1 * l2
    c = l2 * l2 + l3 * l3
    det = a * c - b * b
    Aq, Bq, Cq = 0.5 * (c / det), -b / det, 0.5 * (a / det)
    rx = np.sqrt(2.0 * SIG_CUT * a) + 1.0
    ry = np.sqrt(2.0 * SIG_CUT * c) + 1.0
    featw = features.astype(np.float64) * opacity[:, 0][None, :, None]
    featw = np.transpose(featw, (1, 0, 2)).reshape(N, C * 3)
    return cx, cy, Aq, Bq, Cq, rx, ry, featw


def _host_prep(cx, cy, Aq, Bq, Cq, rx, ry, featw):
    """Bin into 8x16 buckets, split >32 buckets into multiple slots, pack 4
    slots per tile. Returns (w12, feat, slotmap, T):
      w12 [core][12, BPX + T*128]  (phi in the first BPX cols)
      feat [core][128, T*192]      stacked slot-pair feature lhsT
      slotmap [core][T*4] -> bucket index (row*NBC+col) or -1
    """
    h_lo = np.floor(cy - ry).astype(int)
    h_hi = np.ceil(cy + ry).astype(int)
    w_lo = np.floor(cx - rx).astype(int)
    w_hi = np.ceil(cx + rx).astype(int)
    nrow = H // BKH
    buckets = [[[] for _ in range(NBC)] for _ in range(nrow)]
    for n in range(N):
        for bh in range(max(0, h_lo[n] // BKH), min(nrow, h_hi[n] // BKH + 1)):
            for bw in range(max(0, w_lo[n] // BKW), min(NBC, w_hi[n] // BKW + 1)):
                buckets[bh][bw].append(n)

    core_slots = []
    for core in range(NCORES):
        slots = []
        for r in range(NBR):
            for cidx in range(NBC):
                ns = buckets[core * NBR + r][cidx]
                for off in range(0, max(len(ns), 1), SLOT):
                    slots.append((r * NBC + cidx, ns[off:off + SLOT]))
        core_slots.append(slots)
    T = max((len(s) + 3) // 4 for s in core_slots)
    T += T % 2  # pair logic (feat/copy) needs an even tile count

    w12 = np.zeros((NCORES, 12, BPX + T * 128), dtype=np.float16)
    feat = np.zeros((NCORES, 128, T * 192), dtype=np.float16)
    slotmap = np.full((NCORES, T * 4), -1, dtype=np.int32)
    for core in range(NCORES):
        for si, (bid, ns) in enumerate(core_slots[core]):
            ns = np.array(ns, dtype=int)
            k = len(ns)
            slotmap[core, si] = bid
            if k == 0:
                continue
            t, sl = si // 4, si % 4
            r, cidx = bid // NBC, bid % NBC
            cxl = cx[ns] - cidx * BKW - BKW / 2
            cyl = cy[ns] - (core * NBR + r) * BKH - BKH / 2
            An, Bn, Cn = Aq[ns], Bq[ns], Cq[ns]
            W6 = np.stack(
                [
                    An,
                    Cn,
                    Bn,
                    -(2.0 * An * cxl + Bn * cyl),
                    -(2.0 * Cn * cyl + Bn * cxl),
                    An * cxl * cxl + Cn * cyl * cyl + Bn * cxl * cyl,
                ],
                0,
            )
            W_hi = W6.astype(np.float16)
            W_lo = (W6 - W_hi.astype(np.float64)).astype(np.float16)
            base = BPX + t * 128 + sl * SLOT
            w12[core, :6, base:base + k] = W_hi
            w12[core, 6:, base:base + k] = W_lo
            pair = sl // 2
            row0 = 64 * pair + 32 * (sl % 2)
            col0 = t * 192 + 96 * pair + 48 * (sl % 2)
            feat[core, row0:row0 + k, col0:col0 + 48] = featw[ns].astype(
                np.float16)

    xs = (np.arange(BKW) + 0.5 - BKW / 2).astype(np.float32)
    ys = (np.arange(BKH) + 0.5 - BKH / 2).astype(np.float32)
    Yg, Xg = np.meshgrid(ys, xs, indexing="ij")
    phi6 = np.stack(
        [Xg * Xg, Yg * Yg, Xg * Yg, Xg, Yg, np.ones_like(Xg)], 0
    ).reshape(6, BPX)
    w12[:, :, 0:BPX] = np.concatenate([phi6, phi6], 0).astype(np.float16)
    return w12, feat, slotmap, T


def _spans(sizes):
    out, lo = [], 0
    for s in sizes:
        out.append((lo, lo + s))
        lo += s
    return out


def _build_program(T):
    import concourse.bacc as bacc
    import concourse.tile as tile
    import concourse.mybir as mybir

    if T == 16:
        eg, fg = EXP_GROUPS, FLUSH_GROUPS
    else:
        eg = [2] + [4] * ((T - 4) // 4) + [2 + (T - 4) % 4]
        fg = [4] * (T // 4) + ([T % 4] if T % 4 else [])
    assert sum(eg) == T and sum(fg) == T
    egs, fgs = _spans(eg), _spans(fg)

    nc = bacc.Bacc("TRN2", target_bir_lowering=False, debug=False,
                   num_devices=NCORES)
    w12_ap = nc.dram_tensor("w12", [12, BPX + T * 128], mybir.dt.float16,
                            kind="ExternalInput").ap()
    feat_ap = nc.dram_tensor("feat", [128, T * 192], mybir.dt.float16,
                             kind="ExternalInput").ap()
    # out[p, t*256 + pair*128 + px]: p<48 -> channel p of slot 2*pair,
    # p>=48 -> channel p-48 of slot 2*pair+1. Host scatters + upcasts.
    out_ap = nc.dram_tensor("out", [96, T * 256], mybir.dt.float16,
                            kind="ExternalOutput").ap()

    with tile.TileContext(nc) as tc:
        with ExitStack() as ctx:
            consts = ctx.enter_context(tc.tile_pool(name="consts", bufs=1))
            spool = ctx.enter_context(
                tc.tile_pool(name="sig", bufs=2, space="PSUM"))
            opool = ctx.enter_context(
                tc.tile_pool(name="acc", bufs=6, space="PSUM"))
            gpool = ctx.enter_context(tc.tile_pool(name="g", bufs=4))

            # PE p-state warmup: the ramp clock starts at PE's first busy
            # moment; bridge until the w12 DMA lands (~3us)
            dummy = consts.tile([12, 384], mybir.dt.float16)
            nc.gpsimd.memset(dummy, 0)
            for _ in range(8):
                psum_w = spool.tile([128, 4 * BPX], mybir.dt.float32,
                                    name="ps")
                nc.tensor.matmul(psum_w[:, 0:256], dummy[:, 0:128],
                                 dummy[:, 128:384], start=True, stop=True)

            # fp16 staging tiles, one per flush group
            sts = []
            for i, (lo, hi) in enumerate(fgs):
                st = consts.tile([96, (hi - lo) * 256], mybir.dt.float16,
                                 name=f"st{i}")
                sts.append(st)

            # inputs: w12 + feat chunks on the SP HWDGE queue (Pool is busy
            # generating the output scatter descriptors)
            w12_sb = consts.tile([12, BPX + T * 128], mybir.dt.float16)
            nc.sync.dma_start(out=w12_sb, in_=w12_ap)
            phi_sb = w12_sb[:, 0:BPX]
            feat_sb = consts.tile([128, T * 192], mybir.dt.float16)
            FC1, FC2 = 2 * 192, 8 * 192
            nc.sync.dma_start(out=feat_sb[:, :FC1], in_=feat_ap[:, :FC1])
            nc.sync.dma_start(out=feat_sb[:, FC1:FC2], in_=feat_ap[:, FC1:FC2])
            nc.sync.dma_start(out=feat_sb[:, FC2:], in_=feat_ap[:, FC2:])


            psum_s = [None] * len(egs)
            g_sb = [None] * len(egs)
            psum_o = [None] * (T // 2 + 1)

            def sigma(gi):
                lo, hi = egs[gi]
                psum_s[gi] = spool.tile([128, 4 * BPX], mybir.dt.float32,
                                        name="ps")
                for j in range(hi - lo):
                    t = lo + j
                    nc.tensor.matmul(
                        psum_s[gi][:, j * BPX:(j + 1) * BPX],
                        w12_sb[:, BPX + t * 128:BPX + (t + 1) * 128],
                        phi_sb, start=True, stop=True)

            def expg(gi):
                lo, hi = egs[gi]
                n = hi - lo
                g_sb[gi] = gpool.tile([128, 4 * BPX], mybir.dt.float16,
                                      name="g")
                nc.scalar.activation(
                    g_sb[gi][:, 0:n * BPX], psum_s[gi][:, 0:n * BPX],
                    mybir.ActivationFunctionType.Exp, bias=0.0, scale=-1.0)

            def feat(t):
                pi, pj = t // 2, t % 2
                if pj == 0:
                    psum_o[pi] = opool.tile([96, 512], mybir.dt.float32,
                                            name="po")
                gi = next(i for i, (lo, hi) in enumerate(egs) if lo <= t < hi)
                gj = t - egs[gi][0]
                # K=128 with the complementary half of feat_sb zeroed: both
                # slot-pairs run at tile_position (0,0) (off-diagonal PE
                # tiles fault on hardware)
                for pair in range(2):
                    nc.tensor.matmul(
                        psum_o[pi][:, pj * 256 + pair * 128:
                                   pj * 256 + (pair + 1) * 128],
                        feat_sb[:, t * 192 + 96 * pair:
                                t * 192 + 96 * pair + 96],
                        g_sb[gi][:, gj * BPX:(gj + 1) * BPX],
                        start=True, stop=True)

            def copy_pair(pi, eng):
                t = 2 * pi
                fi = next(i for i, (lo, hi) in enumerate(fgs) if lo <= t < hi)
                lo = fgs[fi][0]
                dst = sts[fi][:, (t - lo) * 256:(t - lo + 2) * 256]
                src = psum_o[pi]
                if hasattr(eng, "tensor_copy"):
                    eng.tensor_copy(dst, src)
                else:
                    eng.copy(dst, src)

            # copy-engine plan: DVE takes even-ish tiles, Pool odd, ACT the
            # last tiles once its exp chain is done
            npair = T // 2
            pair_eng = {}
            for pi in range(npair):
                pair_eng[pi] = nc.vector if pi < npair - 3 else nc.scalar

            def flush(i):
                lo, hi = fgs[i]
                nc.sync.dma_start(
                    out=out_ap[:, lo * 256:hi * 256], in_=sts[i])

            fsched = {hi - 1: [i] for i, (lo, hi) in enumerate(fgs)}

            def maybe_flush(t):
                for i in fsched.get(t, []):
                    flush(i)

            # emission order = per-engine program order: all sigmas and
            # exps lead (ACT is the pipeline driver), feats/copies/flushes
            # trail in tile order
            for gi in range(len(egs)):
                sigma(gi)
                expg(gi)
            for t in range(T):
                feat(t)
                if t % 2 == 1:
                    copy_pair(t // 2, pair_eng[t // 2])
                maybe_flush(t)

    nc.compile()
    return nc


def kernel(xyz_raw, cholesky_raw, features, opacity):
    from concourse.bass_utils import run_bass_kernel_spmd

    np_inputs = {
        "xyz_raw": np.asarray(xyz_raw, dtype=np.float32),
        "cholesky_raw": np.asarray(cholesky_raw, dtype=np.float32),
        "features": np.asarray(features, dtype=np.float32),
        "opacity": np.asarray(opacity, dtype=np.float32),
    }
    w12, feat, slotmap, T = _host_prep(*_params(np_inputs))
    if T not in _cached:
        _cached[T] = _build_program(T)
    nc = _cached[T]
    in_maps = [{"w12": w12[b], "feat": feat[b]} for b in range(NCORES)]
    res = run_bass_kernel_spmd(nc, in_maps, core_ids=list(range(NCORES)))

    out = np.zeros((C * 3, H, W), dtype=np.float32)
    for core in range(NCORES):
        r = np.asarray(res.results[core]["out"], dtype=np.float32)
        for si in range(T * 4):
            bid = slotmap[core, si]
            if bid < 0:
                continue
            t, sl = si // 4, si % 4
            pair, half = sl // 2, sl % 2
            blk = r[48 * half:48 * half + 48,
                    t * 256 + pair * 128:t * 256 + (pair + 1) * 128]
            row = core * NBR + bid // NBC
            cw = bid % NBC
            out[:, row * BKH:(row + 1) * BKH, cw * BKW:(cw + 1) * BKW] += \
                blk.reshape(48, BKH, BKW)
    return out.reshape(C, 3, H, W)
